# revision 8
# baseline (speedup 1.0000x reference)
"""Windowed multi-head attention (DWAttention) Bass kernel for Trainium2.

Problem: x[B=2, n=64, N=256, C=384] -> per-window MHA (H=12, d=32) with fused
QKV projection + out_proj (no bias on out_proj, in_proj bias provided).

Strategy (8 NeuronCores, data-parallel over the B*n = 128 independent
windows -> 16 windows per core):

Per window w (tokens N=256, channels C=384 = 3 partition-tiles of 128):
  1. Host supplies x^T [C, N] (layout prep on host, analogous to
     pre-transposed weights).  All matmuls use float32r (full-rate fp32).
  2. qk^T = W_qk @ x^T: 6 psum tiles [128, 256] (chan-major), evicted to
     SBUF with per-partition bias add (DVE tensor_scalar).
  3. v = x @ W_v^T: 2 psum tiles [128, 384] (token-major), evicted with
     broadcast bias add.
  4. Per head-group g of 4 heads (3 groups), per k-tile t (2):
     S^T[k, q] = k_h @ q_h^T via row-group packed matmuls (K=d=32, 4 heads
     concurrent in the 128x128 array) -> psum [128, 4*256].
     exp via ScalarE activation (scale=1/sqrt(d) fused), psum -> SBUF.
  5. attn@v + denominator: col-group packed matmuls (M=32 per head):
     o^T[d, q] accumulates over the 2 k-tiles; denominator rows = ones^T
     matmuls producing the k-sum replicated over each head's 32 partitions.
     Normalize during psum->SBUF eviction: oT_sb = psum_oT * recip(den).
  6. out = o @ W_o^T: lhsT = oT tiles (exactly the c-major layout produced
     in 5), 2 psum tiles [128, 384], evicted and DMA'd out.
"""

import numpy as np
from contextlib import ExitStack

import concourse.bass as bass
import concourse.mybir as mybir
import concourse.tile as tile
from concourse import bacc
from concourse.bass_utils import run_bass_kernel_spmd

# Problem constants (hardcoded per contract).
B, NWIN, N, C = 2, 64, 256, 384
H, D = 12, 32
SCALE = float(D) ** -0.5
NCORES = 8
WPC = (B * NWIN) // NCORES  # windows per core = 16
CT = C // 128               # channel tiles = 3
TT = N // 128               # token tiles = 2
NG = H // 4                 # head groups of 4 = 3

F32 = mybir.dt.float32
F32R = mybir.dt.float32r
BF16 = mybir.dt.bfloat16
ADD = mybir.AluOpType.add
MULT = mybir.AluOpType.mult
EXP = mybir.ActivationFunctionType.Exp


def _r(ap):
    """Bitcast an fp32 AP to float32r for full-rate PE matmuls."""
    return ap.bitcast(F32R)


def build_program(stage=4, wpc=WPC):
    """stage: 1=proj only, 2=+scores/exp, 3=+attnv/recip, 4=full."""
    nc = bacc.Bacc()

    xt_h = nc.dram_tensor("xt", [wpc, CT, 128, N], F32R, kind="ExternalInput")
    wqk_h = nc.dram_tensor("wqkt", [CT, 128, 2 * C], F32R, kind="ExternalInput")
    wv_h = nc.dram_tensor("wvt", [CT, 128, C], F32R, kind="ExternalInput")
    wo_h = nc.dram_tensor("wot", [CT, 128, C], F32R, kind="ExternalInput")
    bqk_h = nc.dram_tensor("bqkt", [128, 2 * CT], F32, kind="ExternalInput")
    bvb_h = nc.dram_tensor("bvb", [128, C], F32, kind="ExternalInput")
    out_h = nc.dram_tensor("out", [wpc, TT, 128, C], F32, kind="ExternalOutput")
    dbg_h = None
    if stage == 1:
        dbg_h = nc.dram_tensor("dbg", [wpc, 128, 2 * CT * N], F32R, kind="ExternalOutput")
    elif stage == 2:
        dbg_h = nc.dram_tensor("dbg", [wpc, NG, 128, 8 * N], BF16, kind="ExternalOutput")
    elif stage == 3:
        dbg_h = nc.dram_tensor("dbg", [wpc, NG, 128, N], F32R, kind="ExternalOutput")

    with ExitStack() as ctx:
        tc = ctx.enter_context(tile.TileContext(nc))
        wpool = ctx.enter_context(tc.tile_pool(name="wpool", bufs=1))
        xpool = ctx.enter_context(tc.tile_pool(name="xpool", bufs=3))
        qkpool = ctx.enter_context(tc.tile_pool(name="qkpool", bufs=2))
        vpool = ctx.enter_context(tc.tile_pool(name="vpool", bufs=2))
        apool = ctx.enter_context(tc.tile_pool(name="apool", bufs=2))
        rpool = ctx.enter_context(tc.tile_pool(name="rpool", bufs=2))
        opool = ctx.enter_context(tc.tile_pool(name="opool", bufs=6))
        fpool = ctx.enter_context(tc.tile_pool(name="fpool", bufs=2))
        proj_ps = ctx.enter_context(tc.tile_pool(name="proj_ps", bufs=2, space="PSUM"))
        sc_ps = ctx.enter_context(tc.tile_pool(name="sc_ps", bufs=1, space="PSUM"))
        att_ps = ctx.enter_context(tc.tile_pool(name="att_ps", bufs=1, space="PSUM"))

        # ---- one-time constants ----
        wqk_sb = wpool.tile([128, CT, 2 * C], F32R)
        nc.sync.dma_start(out=wqk_sb, in_=wqk_h.ap().rearrange("c p o -> p c o"))
        wv_sb = wpool.tile([128, CT, C], F32R)
        nc.sync.dma_start(out=wv_sb, in_=wv_h.ap().rearrange("c p o -> p c o"))
        wo_sb = wpool.tile([128, CT, C], F32R)
        nc.sync.dma_start(out=wo_sb, in_=wo_h.ap().rearrange("c p o -> p c o"))
        bqk_sb = wpool.tile([128, 2 * CT], F32)
        nc.sync.dma_start(out=bqk_sb, in_=bqk_h.ap())
        bvb_sb = wpool.tile([128, C], F32)
        nc.sync.dma_start(out=bvb_sb, in_=bvb_h.ap())
        ones_sb = wpool.tile([128, 32], BF16)
        nc.vector.memset(ones_sb, 1.0)

        for w in range(wpc):
            # ---- load x^T for this window ----
            xt_sb = xpool.tile([128, CT, N], F32R)
            nc.sync.dma_start(out=xt_sb, in_=xt_h.ap()[w].rearrange("c p t -> p c t"))

            # ---- qk^T projection: 6 output chan-tiles of [128, 256] ----
            qk_sb = qkpool.tile([128, 2 * CT, N], BF16)
            for j in range(2 * CT):
                ps = proj_ps.tile([128, N], F32, tag="proj", name="ps_qk")
                for c in range(CT):
                    nc.tensor.matmul(
                        ps,
                        wqk_sb[:, c, 128 * j:128 * (j + 1)],
                        xt_sb[:, c, :],
                        start=(c == 0), stop=(c == CT - 1),
                    )
                nc.vector.tensor_scalar(
                    out=qk_sb[:, j, :], in0=ps,
                    scalar1=bqk_sb[:, j:j + 1], scalar2=None, op0=ADD,
                )

            # ---- v projection: 2 token-tiles of [128, 384] ----
            v_sb = vpool.tile([128, TT, C], BF16)
            for m in range(TT):
                ps = proj_ps.tile([128, C], F32, tag="proj", name="ps_v")
                for c in range(CT):
                    nc.tensor.matmul(
                        ps,
                        xt_sb[:, c, 128 * m:128 * (m + 1)],
                        wv_sb[:, c, :],
                        start=(c == 0), stop=(c == CT - 1),
                    )
                nc.vector.tensor_tensor(
                    out=v_sb[:, m, :], in0=ps, in1=bvb_sb, op=ADD,
                )

            if stage == 1:
                nc.sync.dma_start(out=dbg_h.ap()[w], in_=qk_sb.rearrange("p j n -> p (j n)"))
                continue

            # ---- attention per head-group of 4 ----
            ot_tiles = []
            for g in range(NG):
                attn_sb = apool.tile([128, 2 * 4 * N], BF16, name="attn_sb")
                for t in range(TT):
                    # one PSUM bank per matmul group (HW requirement): 4 banks
                    scp = sc_ps.tile([128, 4, 512], F32, name="scp")
                    for hh in range(4):
                        # S^T[k_slice, q] = k_h[k_slice] @ q_h^T
                        nc.tensor.matmul(
                            scp[:, hh, 0:N],
                            qk_sb[32 * hh:32 * (hh + 1), CT + g, 128 * t:128 * (t + 1)],
                            qk_sb[32 * hh:32 * (hh + 1), g, :],
                            start=True, stop=True,
                            tile_position=(32 * hh, 0),
                        )
                    nc.scalar.activation(
                        out=attn_sb[:, 4 * N * t:4 * N * (t + 1)], in_=scp[:, :, 0:N],
                        func=EXP, scale=SCALE,
                    )

                if stage == 2:
                    nc.sync.dma_start(out=dbg_h.ap()[w][g], in_=attn_sb)
                    continue

                # bank 0: o^T accumulation, bank 1: denominators (one bank per group)
                od = att_ps.tile([128, 2, 512], F32, name="od")
                for t in range(TT):
                    for hh in range(4):
                        h = 4 * g + hh
                        nc.tensor.matmul(
                            od[32 * hh:32 * (hh + 1), 0, 0:N],
                            v_sb[:, t, 32 * h:32 * (h + 1)],
                            attn_sb[:, 4 * N * t + N * hh: 4 * N * t + N * (hh + 1)],
                            start=(t == 0), stop=(t == TT - 1),
                            tile_position=(0, 32 * hh),
                            skip_group_check=True,
                        )
                for t in range(TT):
                    for hh in range(4):
                        nc.tensor.matmul(
                            od[32 * hh:32 * (hh + 1), 1, 0:N],
                            ones_sb[:, :],
                            attn_sb[:, 4 * N * t + N * hh: 4 * N * t + N * (hh + 1)],
                            start=(t == 0), stop=(t == TT - 1),
                            tile_position=(0, 32 * hh),
                            skip_group_check=True,
                        )
                recip_sb = rpool.tile([128, N], F32)
                nc.vector.reciprocal_approx_fast(recip_sb, od[:, 1, 0:N])
                ot_sb = opool.tile([128, N], F32R, name="ot_sb")
                nc.vector.tensor_tensor(out=ot_sb, in0=od[:, 0, 0:N], in1=recip_sb, op=MULT)
                ot_tiles.append(ot_sb)
                if stage == 3:
                    nc.sync.dma_start(out=dbg_h.ap()[w][g], in_=ot_sb)

            if stage in (2, 3):
                continue

            # ---- out projection ----
            of_sb = fpool.tile([128, TT, C], F32)
            for m in range(TT):
                ps = proj_ps.tile([128, C], F32, tag="proj", name="ps_out")
                for g in range(NG):
                    nc.tensor.matmul(
                        ps,
                        ot_tiles[g][:, 128 * m:128 * (m + 1)],
                        wo_sb[:, g, :],
                        start=(g == 0), stop=(g == NG - 1),
                    )
                if m == 0:
                    nc.scalar.copy(out=of_sb[:, m, :], in_=ps)
                else:
                    nc.vector.tensor_copy(out=of_sb[:, m, :], in_=ps)
            nc.sync.dma_start(out=out_h.ap()[w].rearrange("m p c -> p m c"), in_=of_sb)

    nc.compile()
    return nc


_PROGRAM = None


def _get_program():
    global _PROGRAM
    if _PROGRAM is None:
        _PROGRAM = build_program()
    return _PROGRAM


def make_in_maps(x, in_proj_weight, in_proj_bias, out_proj_weight):
    x = np.asarray(x, dtype=np.float32)
    in_proj_weight = np.asarray(in_proj_weight, dtype=np.float32)
    in_proj_bias = np.asarray(in_proj_bias, dtype=np.float32)
    out_proj_weight = np.asarray(out_proj_weight, dtype=np.float32)

    xt = np.ascontiguousarray(x.reshape(B * NWIN, N, C).transpose(0, 2, 1))
    xt = xt.reshape(NCORES, WPC, CT, 128, N)
    wqkt = np.ascontiguousarray(in_proj_weight[:2 * C].T).reshape(CT, 128, 2 * C)
    wvt = np.ascontiguousarray(in_proj_weight[2 * C:].T).reshape(CT, 128, C)
    wot = np.ascontiguousarray(out_proj_weight.T).reshape(CT, 128, C)
    bqkt = np.ascontiguousarray(in_proj_bias[:2 * C].reshape(2 * CT, 128).T)
    bvb = np.ascontiguousarray(np.broadcast_to(in_proj_bias[2 * C:], (128, C)))
    return [
        {"xt": xt[i], "wqkt": wqkt, "wvt": wvt, "wot": wot, "bqkt": bqkt, "bvb": bvb}
        for i in range(NCORES)
    ]


def assemble_out(results):
    outs = [r["out"].reshape(WPC, N, C) for r in results]
    return np.concatenate(outs).reshape(B, NWIN, N, C).astype(np.float32)


def kernel(x, in_proj_weight, in_proj_bias, out_proj_weight):
    nc = _get_program()
    in_maps = make_in_maps(x, in_proj_weight, in_proj_bias, out_proj_weight)
    res = run_bass_kernel_spmd(nc, in_maps, core_ids=list(range(NCORES)))
    return assemble_out(res.results)
